# revision 42
# baseline (speedup 1.0000x reference)
"""Multi-head causal self-attention (B=2, T=2048, D=1024, H=16) on 8 trn2 cores.

Sharding: core c handles batch b=c//4 and head-group g=c%4 (4 heads, 256 feats).
The kernel streams token supertiles of 512: for each ts it computes Q/K/V for
the core's 4 heads, runs causal attention for query block qs=ts (K/V for all
needed kv blocks are already resident -- causal), multiplies the local 256
attention features into a partial output [1024, 512] against w_o rows, and
ReduceScatters that partial (bf16, op=add) across the 4 cores of the batch
group -- each core receives its own 256 output features for those 512 tokens.
The 4 chunked ReduceScatters overlap the next supertile's compute, replacing
the two serialized full-width AllGathers of the tensor-parallel formulation.

Q/K/S matmuls run in fp32r (full-rate PE); exp(S), V and the w_o partials are
bf16 (the reduce accumulates exactly in the collective's add). Softmax skips
the running max (scores bounded ~N(0,1) by construction); the 1/sqrt(Dh)
scale is folded into the Exp activation; the denominator comes from a
ones-column appended to V. Scores are computed as S^T[k, q] so softmax
reduces along the free axis and P@V consumes exp(S^T) directly -- no
transposes. The two heads of an f-tile sit at partition bases 0/64 so their
K=64 score matmuls occupy disjoint PE row-groups; their exps fuse into one
1024-column ACT op; S of block kb+1 issues before PV of block kb.

Attention is ACT-bound (exp) while QKV/w_o are PE-bound, so emission WEAVES
them: each attention generator is interleaved, a few matmul groups per score
block, with the previous supertile's w_o groups (fast rate, so its
ReduceScatter issues mid-attention) and the next supertile's QKV groups.
Q/K/V biases ride as rank-1 accumulate matmuls so psum drains are pure ACT
copies and the DVE stays free for the softmax-normalize chain; w_o epilogues
split half/half across ACT and DVE. All loads are single strided DMAs (HWDGE
is paced at ~625 ns per descriptor-gen) ordered by first use; warm-up matmuls
on the resident mask tile ramp the PE clock under the DMA lead-in. b_o/4 is
added to each partial pre-reduce, so the RS output is final (host just
transposes/concats and casts bf16->fp32).
"""

import os
import sys

for _p in ("/opt/trn_rl_repo", "/root/.axon_site/_ro/trn_rl_repo"):
    if os.path.isdir(_p) and _p not in sys.path:
        sys.path.insert(0, _p)

import numpy as np

import concourse.bacc as bacc
import concourse.mybir as mybir
import concourse.tile as tile
from concourse.bass_utils import run_bass_kernel_spmd

F32 = mybir.dt.float32
F32R = mybir.dt.float32r
BF16 = mybir.dt.bfloat16
AF = mybir.ActivationFunctionType

B, T, C = 2, 2048, 1024
H, Dh = 16, 64
NCORES, GRP = 8, 4        # 2 groups of 4 cores (one per batch)
HL, FL = 4, 256           # heads / features per core
TS = 512                  # token supertile
NQ = T // TS              # 4
JL = C // GRP             # 256 output features per core after the RS

_CACHE = {}
_TRACE = False
_LAST = None


def _build(unroll=1, with_bias=True):
    nc = bacc.Bacc("TRN2", target_bir_lowering=False, debug=False,
                   num_devices=NCORES)

    xT = nc.dram_tensor("xT", [C, T], F32R, kind="ExternalInput")
    wqT = nc.dram_tensor("wqT", [C, FL], F32R, kind="ExternalInput")
    wkT = nc.dram_tensor("wkT", [C, FL], F32R, kind="ExternalInput")
    wvT = nc.dram_tensor("wvT", [C, FL], F32R, kind="ExternalInput")
    woL = nc.dram_tensor("woL", [FL, C], F32R, kind="ExternalInput")
    bqk_row = nc.dram_tensor("bqk_row", [1, 4, 128], F32R, kind="ExternalInput")
    ones_in = nc.dram_tensor("ones_in", [128, 64], BF16, kind="ExternalInput")
    ones_rin = nc.dram_tensor("ones_rin", [1, TS], F32R, kind="ExternalInput")
    xT0_bf = nc.dram_tensor("xT0_bf", [C, TS], BF16, kind="ExternalInput")
    wqkv_bf = nc.dram_tensor("wqkv_bf", [C, 3 * FL], BF16, kind="ExternalInput")
    bv_row = nc.dram_tensor("bv_row", [1, FL], F32R, kind="ExternalInput")
    bo_bc = nc.dram_tensor("bo_bc", [128, 8], F32, kind="ExternalInput")
    mask2 = nc.dram_tensor("mask2", [128, 2, 128], BF16, kind="ExternalInput")
    outRS = nc.dram_tensor("outRS", [NQ, JL, TS], BF16, kind="ExternalOutput")

    with tile.TileContext(nc) as tc:
        for _it in range(unroll):
            with tc.tile_pool(name="persist", bufs=1) as pp:
                # ---- persistent SBUF state ----
                QT = pp.tile([128, 2, T], F32R)          # Q^T  [f, t]
                KT = pp.tile([128, 2, T], F32R)          # K^T  [f, t]
                Vg = pp.tile([128, T // 128, HL, Dh + 1], BF16)
                attnT = pp.tile([128, 2, T], F32R)       # attention out^T
                mask_sb = pp.tile([128, 2, 128], BF16)
                bqkr_sb = pp.tile([1, 4, 128], F32R)
                bvr_sb = pp.tile([1, FL], F32R)
                bo_sb = pp.tile([128, 8], F32)
                ones_sb = pp.tile([128, 64], BF16)
                ones_row = pp.tile([1, TS], F32R)

                nc.sync.dma_start(mask_sb[:], mask2[:])
                # PE warm-up ramps the HAM clock under the input-DMA lead-in;
                # psum is discarded.
                with tc.tile_pool(name="warm", bufs=1, space="PSUM") as wp:
                    ps_w = wp.tile([128, 256], F32, name="ps_w")
                    for _w in range(52):
                        nc.tensor.matmul(
                            ps_w[:], lhsT=mask_sb[:, 0, :],
                            rhs=mask_sb.rearrange("p a b -> p (a b)"),
                            start=True, stop=True)

                dp = tc.tile_pool(name="dram", bufs=1, space="DRAM")
                dpp = dp.__enter__()
                # one tile per chunk: a shared tile would add false
                # whole-tile hazards serializing each RS behind the previous
                rs_in = [dpp.tile([C, TS], BF16, name=f"rs_in{i}")
                         for i in range(NQ)]
                rs_out = [dpp.tile([JL, TS], BF16, name=f"rs_out{i}")
                          for i in range(NQ)]

                with tc.tile_pool(name="xw", bufs=1) as xw, \
                     tc.tile_pool(name="att", bufs=3) as att, \
                     tc.tile_pool(name="fin2", bufs=2) as fin2, \
                     tc.tile_pool(name="psA", bufs=2, space="PSUM") as psA, \
                     tc.tile_pool(name="psS", bufs=2, space="PSUM") as psS, \
                     tc.tile_pool(name="psO", bufs=2, space="PSUM") as psO:
                    xT_sb = xw.tile([128, 8, T - TS], F32R)
                    x0_bf = xw.tile([128, 8, TS], BF16)
                    wqkv_sb = xw.tile([128, 8, 3 * FL], BF16)
                    wq_sb = xw.tile([128, 8, FL], F32R)
                    wk_sb = xw.tile([128, 8, FL], F32R)
                    wv_sb = xw.tile([128, 8, FL], F32R)
                    wo_sb = xw.tile([128, 2, C], F32R)
                    # coalesced loads, one strided DMA each (HWDGE is paced
                    # at ~625 ns per DMACopy), ordered by first use
                    xTr = xT.rearrange("(c p) t -> p c t", p=128)
                    x0r = xT0_bf.rearrange("(c p) t -> p c t", p=128)
                    # supertile 0's QKV inputs ride in bf16: half the DMA
                    # bytes on the critical startup path
                    nc.sync.dma_start(x0_bf[:, 0:4, :], x0r[:, 0:4, :])
                    nc.sync.dma_start(
                        wqkv_sb[:], wqkv_bf.rearrange("(c p) f -> p c f", p=128))
                    nc.sync.dma_start(bqkr_sb[:], bqk_row[:])
                    nc.sync.dma_start(ones_row[:], ones_rin[:])
                    nc.sync.dma_start(x0_bf[:, 4:8, :], x0r[:, 4:8, :])
                    nc.sync.dma_start(bvr_sb[:], bv_row[:])
                    nc.sync.dma_start(ones_sb[:], ones_in[:])
                    nc.sync.dma_start(
                        wq_sb[:], wqT.rearrange("(c p) f -> p c f", p=128))
                    # softmax denominator column of V via a strided ACT copy
                    nc.scalar.activation(
                        Vg[:, :, :, Dh:Dh + 1],
                        ones_sb.rearrange("p (a b o) -> p a b o", a=T // 128,
                                          b=HL), AF.Copy)
                    nc.sync.dma_start(
                        wk_sb[:], wkT.rearrange("(c p) f -> p c f", p=128))
                    nc.sync.dma_start(
                        wv_sb[:], wvT.rearrange("(c p) f -> p c f", p=128))
                    nc.sync.dma_start(xT_sb[:, :, 0:TS], xTr[:, :, TS:2 * TS])
                    nc.sync.dma_start(
                        wo_sb[:], woL.rearrange("(c p) j -> p c j", p=128))
                    nc.sync.dma_start(bo_sb[:], bo_bc[:])
                    for ts_ in range(2, NQ):
                        nc.sync.dma_start(
                            xT_sb[:, :, (ts_ - 1) * TS:ts_ * TS],
                            xTr[:, :, ts_ * TS:(ts_ + 1) * TS])

                    def qkv_gen(ts_, do_qk=True, do_v=True):
                        bf = ts_ == 0
                        # Q^T,K^T [f, t] for both f-tiles of this supertile.
                        # Biases ride as an extra rank-1 accumulate matmul so
                        # the psum drain is a pure ACT copy -- DVE stays free
                        # for the softmax-normalize chain.
                        for wi, (dst, w_sb, bcol) in enumerate(
                                ((QT, wq_sb, 0), (KT, wk_sb, 2)) if do_qk
                                else ()):
                            for ft in range(2):
                                ps = psA.tile([128, TS], F32, name="ps_qk",
                                              tag="psA")
                                if with_bias:
                                    nc.tensor.matmul(
                                        ps[:], lhsT=bqkr_sb[0:1, bcol + ft, :],
                                        rhs=ones_row[:], start=True, stop=False)
                                for cc in range(8):
                                    if bf:
                                        lhs = wqkv_sb[:, cc,
                                                      wi * FL + ft * 128:
                                                      wi * FL + (ft + 1) * 128]
                                        rhs = x0_bf[:, cc, :]
                                    else:
                                        lhs = w_sb[:, cc,
                                                   ft * 128:(ft + 1) * 128]
                                        rhs = xT_sb[:, cc,
                                                    (ts_ - 1) * TS:ts_ * TS]
                                    nc.tensor.matmul(
                                        ps[:], lhsT=lhs, rhs=rhs,
                                        start=(not with_bias and cc == 0),
                                        stop=(cc == 7))
                                nc.scalar.activation(
                                    dst[:, ft, ts_ * TS:(ts_ + 1) * TS], ps[:],
                                    AF.Copy)
                                yield
                        # V token-major: [t, f] = sum_c x^T[c, t] w_v^T[c, f]
                        for tb in (range(4 * ts_, 4 * ts_ + 4) if do_v
                                   else ()):
                            ps = psA.tile([128, TS], F32, name="ps_v",
                                          tag="psA")[:, :FL]
                            if with_bias:
                                nc.tensor.matmul(
                                    ps[:], lhsT=ones_row[0:1, 0:128],
                                    rhs=bvr_sb[:], start=True, stop=False)
                            for cc in range(8):
                                if bf:
                                    lhs = x0_bf[:, cc,
                                                (tb % 4) * 128:(tb % 4 + 1) * 128]
                                    rhs = wqkv_sb[:, cc, 2 * FL:3 * FL]
                                else:
                                    lhs = xT_sb[:, cc, (tb - 4) * 128:
                                                (tb - 3) * 128]
                                    rhs = wv_sb[:, cc, :]
                                nc.tensor.matmul(
                                    ps[:], lhsT=lhs, rhs=rhs,
                                    start=(not with_bias and cc == 0),
                                    stop=(cc == 7))
                            nc.scalar.activation(
                                Vg[:, tb, :, 0:Dh],
                                ps.rearrange("p (h d) -> p h d", h=HL),
                                AF.Copy)
                            yield

                    def att_gen(qs, split_last=False):
                        # all 4 heads for query supertile qs; heads (2ft, 2ft+1)
                        # at partition bases (0, 64)
                        for ft in range(2):
                            Q0, K0 = QT[0:64, ft, :], KT[0:64, ft, :]
                            Q1, K1 = QT[64:128, ft, :], KT[64:128, ft, :]
                            h0, h1 = 2 * ft, 2 * ft + 1
                            po0 = psO.tile([128, TS], F32, name="po0",
                                           tag="ps_o")
                            po1 = psO.tile([128, TS], F32, name="po1",
                                           tag="ps_o")
                            nkb = 4 * qs + 4

                            def s_part(kb, q_lo):
                                # S^T for both heads at kv block kb -> exp
                                ps_s = psS.tile([128, 2, TS], F32,
                                                name="ps_s", tag="ps_s")
                                nc.tensor.matmul(
                                    ps_s[:, 0, q_lo:TS],
                                    lhsT=K0[:, kb * 128:(kb + 1) * 128],
                                    rhs=Q0[:, qs * TS + q_lo:(qs + 1) * TS],
                                    start=True, stop=True)
                                nc.tensor.matmul(
                                    ps_s[:, 1, q_lo:TS],
                                    lhsT=K1[:, kb * 128:(kb + 1) * 128],
                                    rhs=Q1[:, qs * TS + q_lo:(qs + 1) * TS],
                                    start=True, stop=True)
                                p_sb = att.tile([128, 2, TS], BF16,
                                                name="p_sb", tag="p")
                                nc.scalar.activation(
                                    p_sb[:, :, q_lo:TS], ps_s[:, :, q_lo:TS],
                                    AF.Exp, scale=0.125)
                                diag = kb - 4 * qs
                                if diag >= 0:  # triangular mask
                                    mo = diag * 128
                                    nc.vector.tensor_mul(
                                        p_sb[:, :, mo:mo + 128],
                                        p_sb[:, :, mo:mo + 128],
                                        mask_sb[:])
                                return p_sb

                            def pv_part(kb, q_lo, p_sb):
                                nc.tensor.matmul(
                                    po0[0:65, q_lo:TS],
                                    lhsT=Vg[:, kb, h0, :],
                                    rhs=p_sb[:, 0, q_lo:TS],
                                    start=(kb == 0), stop=(kb == nkb - 1))
                                nc.tensor.matmul(
                                    po1[0:65, q_lo:TS],
                                    lhsT=Vg[:, kb, h1, :],
                                    rhs=p_sb[:, 1, q_lo:TS],
                                    start=(kb == 0), stop=(kb == nkb - 1))

                            # one-block software pipeline: S(kb+1) issues
                            # before PV(kb) so PE never waits on the Exp
                            qlo = lambda kb: max(0, (kb - 4 * qs) * 128)
                            def norm(po, fb, c0, c1):
                                # normalize cols [c0:c1) by the ones-row sums
                                r_sb = att.tile([1, TS], F32R, name="r_sb",
                                                tag="r")[:, c0:c1]
                                with nc.allow_low_precision(reason="f32r"):
                                    nc.vector.reciprocal(r_sb,
                                                         po[64:65, c0:c1])
                                r_bc = att.tile([64, TS], F32R, name="r_bc",
                                                tag="r_bc", bufs=2)[:, c0:c1]
                                nc.gpsimd.partition_broadcast(r_bc, r_sb)
                                nc.vector.tensor_mul(
                                    attnT[fb:fb + 64, ft,
                                          qs * TS + c0:qs * TS + c1],
                                    po[0:64, c0:c1], r_bc)

                            # split: cols [0:256) of the denominator are final
                            # once the j=1 diagonal block's PV has landed, two
                            # blocks before the end -- the early half-normalize
                            # lets wo's first half start under the rest
                            split = split_last and ft == 1
                            prev = None
                            for kb in range(nkb):
                                cur = (kb, qlo(kb), s_part(kb, qlo(kb)))
                                if prev is not None:
                                    pv_part(*prev)
                                    if split and prev[0] == 4 * qs + 1:
                                        norm(po0, 0, 0, TS // 2)
                                        norm(po1, 64, 0, TS // 2)
                                prev = cur
                                yield
                            pv_part(*prev)
                            if split:
                                norm(po0, 0, TS // 2, TS)
                                norm(po1, 64, TS // 2, TS)
                            else:
                                norm(po0, 0, 0, TS)
                                norm(po1, 64, 0, TS)
                            yield

                    def wo_gen(ts_, deep=False):
                        # partial out^T[j, t] = w_o[j, f_local] attnT[f_local, t]
                        # (+ b_o/4) -> bf16 -> RS input; epilogues alternate
                        # ACT/DVE so neither engine paces the psum recycling
                        if deep:
                            o_lo = fin2.tile([128, 8, TS // 2], BF16,
                                             name="o_lo", tag="o")
                            o_hi = fin2.tile([128, 8, TS // 2], BF16,
                                             name="o_hi", tag="o2")
                        else:
                            o_sb = fin2.tile([128, 8, TS], BF16, name="o_sb",
                                             tag="o")
                        halves = (1, 2) if deep else (0,)
                        for h in halves:
                            for jt in range(8):
                                hw0 = 0 if h == 0 else (h - 1) * (TS // 2)
                                hw1 = TS if h == 0 else h * (TS // 2)
                                if deep and jt % 2:
                                    ps = psO.tile([128, TS], F32, name="po0",
                                                  tag="ps_o")[:, hw0:hw1]
                                else:
                                    ps = psA.tile([128, TS], F32, name="ps_f",
                                                  tag="psA")[:, hw0:hw1]
                                for fc in range(2):
                                    nc.tensor.matmul(
                                        ps,
                                        lhsT=wo_sb[:, fc,
                                                   jt * 128:(jt + 1) * 128],
                                        rhs=attnT[:, fc, ts_ * TS + hw0:
                                                  ts_ * TS + hw1],
                                        start=(fc == 0), stop=(fc == 1))
                                # epilogues split across ACT and DVE keep the
                                # psum recycle faster than the matmuls
                                hw = hw1 - hw0
                                dst = (o_sb[:, jt, hw0:hw1] if not deep else
                                       (o_lo if h == 1 else o_hi)[:, jt, :])
                                with nc.allow_low_precision(
                                        reason="bf16 partial"):
                                    nc.scalar.activation(
                                        dst[:, 0:hw // 2], ps[:, 0:hw // 2],
                                        AF.Identity, bias=bo_sb[:, jt:jt + 1])
                                    nc.vector.tensor_scalar_add(
                                        dst[:, hw // 2:hw],
                                        ps[:, hw // 2:hw],
                                        bo_sb[:, jt:jt + 1])
                                if deep and h == 1 and jt == 7:
                                    # low columns are complete: stream them to
                                    # the RS input under the second half-pass
                                    nc.sync.dma_start(
                                        rs_in[ts_].rearrange(
                                            "(j p) t -> p j t", p=128)
                                        [:, :, 0:TS // 2], o_lo[:])
                                if not (h == halves[-1] and jt == 7):
                                    yield
                        rsv = rs_in[ts_].rearrange("(j p) t -> p j t", p=128)
                        if deep:
                            nc.sync.dma_start(rsv[:, :, TS // 2:TS], o_hi[:])
                        else:
                            nc.sync.dma_start(rsv[:, 0:4, :], o_sb[:, 0:4, :])
                            nc.sync.dma_start(rsv[:, 4:8, :], o_sb[:, 4:8, :])
                        nc.gpsimd.collective_compute(
                            "ReduceScatter", mybir.AluOpType.add,
                            replica_groups=[[0, 1, 2, 3], [4, 5, 6, 7]],
                            ins=[rs_in[ts_].opt()], outs=[rs_out[ts_].opt()])

                    def drain(g):
                        for _ in g:
                            pass

                    def weave(main, plans):
                        # plans: [gen, rate] -- advance gen `rate` steps per
                        # main step. The ACT-bound attention's exp chain hides
                        # under the fillers' PE-only matmul groups; wo fillers
                        # run at a high rate so their ReduceScatter issues
                        # mid-attention instead of at the drain.
                        plans = [[g, r, 0.0] for g, r in plans]
                        for _ in main:
                            for p in plans:
                                p[2] += p[1]
                                while p[2] >= 1.0:
                                    p[2] -= 1.0
                                    try:
                                        next(p[0])
                                    except StopIteration:
                                        p[2] = -1e9
                        for p in plans:
                            drain(p[0])

                    drain(qkv_gen(0))
                    weave(att_gen(0), [(qkv_gen(1), 1.2)])
                    weave(att_gen(1), [(wo_gen(0), 0.5), (qkv_gen(2), 0.67)])
                    weave(att_gen(2), [(wo_gen(1), 0.5), (qkv_gen(3), 0.46)])
                    weave(att_gen(3, split_last=True), [(wo_gen(2), 0.5)])
                    drain(wo_gen(3, deep=True))
                    # output copies last: a copy's wait on its RS would block
                    # SP.SEQ and delay later rs_in stages if emitted inline
                    for ts_ in range(NQ):
                        nc.sync.dma_start(outRS[ts_], rs_out[ts_][:])
                dp.__exit__(None, None, None)

    nc.compile()
    return nc


def _bf(a):
    import ml_dtypes
    return np.asarray(a, dtype=ml_dtypes.bfloat16)


def _make_in_maps(x, w_q, b_q, w_k, b_k, w_v, b_v, w_o, b_o):
    x = np.asarray(x, dtype=np.float32)
    w_q = np.asarray(w_q, dtype=np.float32)
    w_k = np.asarray(w_k, dtype=np.float32)
    w_v = np.asarray(w_v, dtype=np.float32)
    w_o = np.asarray(w_o, dtype=np.float32)
    b_q = np.asarray(b_q, dtype=np.float32)
    b_k = np.asarray(b_k, dtype=np.float32)
    b_v = np.asarray(b_v, dtype=np.float32)
    b_o = np.asarray(b_o, dtype=np.float32)

    mask_t = np.triu(np.ones((128, 128), dtype=np.float32))
    xTs = [np.ascontiguousarray(x[b].T) for b in range(B)]
    bo_t = np.ascontiguousarray((b_o / GRP).reshape(8, 128).T)

    in_maps = []
    for c in range(NCORES):
        b, g = c // GRP, c % GRP
        fsl = slice(g * FL, (g + 1) * FL)
        bqk_r = np.concatenate([b_q[fsl].reshape(2, 128),
                                b_k[fsl].reshape(2, 128)])[None]  # [1, 4, 128]
        in_maps.append({
            "xT": xTs[b],
            "wqT": np.ascontiguousarray(w_q[fsl, :].T),
            "wkT": np.ascontiguousarray(w_k[fsl, :].T),
            "wvT": np.ascontiguousarray(w_v[fsl, :].T),
            "woL": np.ascontiguousarray(w_o[:, fsl].T),
            "bqk_row": np.ascontiguousarray(bqk_r),
            "bv_row": np.ascontiguousarray(b_v[fsl][None]),
            "bo_bc": bo_t,
            "mask2": _bf(np.ascontiguousarray(
                np.repeat(mask_t[:, None, :], 2, axis=1))),
            "ones_in": _bf(np.ones((128, 64), dtype=np.float32)),
            "ones_rin": np.ones((1, TS), dtype=np.float32),
            "xT0_bf": _bf(xTs[b][:, 0:TS]),
            "wqkv_bf": _bf(np.concatenate(
                [w_q[fsl, :].T, w_k[fsl, :].T, w_v[fsl, :].T], axis=1)),
        })
    return in_maps


def kernel(x, w_q, b_q, w_k, b_k, w_v, b_v, w_o, b_o):
    global _LAST
    wb = bool(np.any(np.asarray(b_q)) or np.any(np.asarray(b_k))
              or np.any(np.asarray(b_v)))
    key = f"nc{int(wb)}"
    if key not in _CACHE:
        _CACHE[key] = _build(with_bias=wb)
    nc = _CACHE[key]

    in_maps = _make_in_maps(x, w_q, b_q, w_k, b_k, w_v, b_v, w_o, b_o)

    res = run_bass_kernel_spmd(nc, in_maps, core_ids=list(range(NCORES)),
                               trace=_TRACE)
    _LAST = res

    out = np.empty((B, T, C), dtype=np.float32)
    for c in range(NCORES):
        b, g = c // GRP, c % GRP
        o = np.asarray(res.results[c]["outRS"], dtype=np.float32)
        for ts_ in range(NQ):
            out[b, ts_ * TS:(ts_ + 1) * TS, g * JL:(g + 1) * JL] = o[ts_].T
    return out
